# revision 2
# baseline (speedup 1.0000x reference)
"""AnomalyMapGenerator (retrieval kNN) Trainium2 kernel.

reference:  d = sqrt(distance[B, HW, M]); v = 3 smallest of d per row;
            w = softmax(-v); s = w0*v0 -> [B, 56, 56]
            -> bilinear resize to 224x224 -> gaussian blur (sigma=4, reflect).

Strategy (8 NeuronCores, data-parallel over batch, 2 images per core):
  - per core, rows r = b*3136 + hw (6272 rows of 4096 distances).
    Row->SBUF mapping r = 56*p + t over 112 partitions x 56 columns:
    112 descriptors/transfer = exactly 7 per SDMA engine, and the
    per-image [56, 56] score map lands DIRECTLY in SBUF partitions
    [56i, 56i+56) x free t=w -- no DRAM round-trip before the matmuls.
  - main loop (26 pair units of [112, 2x4096] f32, 32 KiB/descriptor):
      DMA load -> ScalarE negate (bf16 out) -> VectorE max8 in bf16
      (top-8 of -d = 3 smallest of d; bf16 halves DVE time + SBUF reads;
      value error ~0.2% << 2e-2 tolerance).
  - softmin runs INCREMENTALLY in column blocks behind the stream; only
    columns 52-55 are processed in the tail.
  - stream end: dedicated tiles (no buffer-recycle semaphore stalls) and
    a chunk taper (2048.. down to 256) so the last-arriving bytes need
    only a tiny negate+max8 before the tail chain.
  - post: resize+blur = linear operator A [224, 56]; both images in ONE
    K=112 bf16 matmul via block-diagonal amat2 [112, 448] (bf16 PE is
    4x fp32), then per-(image, even/odd-row-half) K=56 matmuls; output
    rows ho = 2*hp + c so each image writes 1792 B-contiguous
    descriptors. All matmul operands live at base partition 0.
"""
import os
import numpy as np

B, HW, M = 16, 3136, 4096
IMG_IN, IMG_OUT, SIGMA = 56, 224, 4.0
N_CORES = 8
BPC = B // N_CORES            # images per core
ROWS = BPC * HW               # 6272
P = 112                       # partitions; row r = 56p + t
T = ROWS // P                 # 56 columns

# SDMA engine n of a transfer handles the n-th contiguous chunk of
# ceil(D/16) descriptors (measured).  Engine 15 is intermittently slower
# than the rest, so a few pair units are issued as a [0:105) transfer
# (engines 0-14, 7 descriptors each) plus a [105:112) transfer
# (7 descriptors -> engines 0-6), shifting those bytes off engine 15.
SPLIT_UNITS = {9, 18}

# softmin blocks emitted mid-stream after the covering unit's max8s
BLOCKS = {6: (0, 14), 13: (14, 28), 20: (28, 42), 25: (42, 52)}

_CACHE = {}


def _resize_mat(in_size: int, out_size: int) -> np.ndarray:
    # jax.image.resize(method='bilinear') upsampling weight matrix [out, in]
    scale = out_size / in_size
    sample_f = (np.arange(out_size, dtype=np.float64) + 0.5) / scale - 0.5
    x = np.abs(sample_f[None, :] - np.arange(in_size, dtype=np.float64)[:, None])
    w = np.maximum(0.0, 1.0 - x)
    total = w.sum(axis=0, keepdims=True)
    w = np.where(np.abs(total) > 1e-8, w / total, 0.0)
    ob = (sample_f < -0.5) | (sample_f > in_size - 0.5)
    w[:, ob] = 0.0
    return w.T


def _gauss_mat(n: int, sigma: float) -> np.ndarray:
    # 1D gaussian conv with reflect padding as a matrix [n, n]
    ksize = 2 * int(4.0 * sigma + 0.5) + 1
    xs = np.arange(ksize, dtype=np.float64) - ksize // 2
    g = np.exp(-(xs * xs) / (2.0 * sigma * sigma))
    g = g / g.sum()
    pad = ksize // 2
    Gm = np.zeros((n, n), dtype=np.float64)
    for o in range(n):
        for k in range(ksize):
            idx = o - pad + k
            if idx < 0:
                idx = -idx
            elif idx > n - 1:
                idx = 2 * (n - 1) - idx
            Gm[o, idx] += g[k]
    return Gm


def _amat2() -> np.ndarray:
    import ml_dtypes
    A = _gauss_mat(IMG_OUT, SIGMA) @ _resize_mat(IMG_IN, IMG_OUT)  # [224, 56]
    at = A.T.astype(np.float32)                                    # [56, 224]
    m = np.zeros((P, 2 * IMG_OUT), dtype=np.float32)
    m[0:IMG_IN, 0:IMG_OUT] = at
    m[IMG_IN:P, IMG_OUT:2 * IMG_OUT] = at
    return np.ascontiguousarray(m.astype(ml_dtypes.bfloat16))


def _build():
    from contextlib import ExitStack
    import concourse.bass as bass
    import concourse.tile as tile
    from concourse import bacc, mybir

    f32 = mybir.dt.float32
    bf16 = mybir.dt.bfloat16
    AF = mybir.ActivationFunctionType
    ALU = mybir.AluOpType

    nc = bacc.Bacc("TRN2", target_bir_lowering=False, debug=False,
                   enable_asserts=False)
    dist = nc.dram_tensor("distance", [ROWS, M], f32, kind="ExternalInput")
    amat2 = nc.dram_tensor("amat2", [P, 2 * IMG_OUT], bf16, kind="ExternalInput")
    out = nc.dram_tensor("out", [BPC, IMG_OUT, IMG_OUT], f32, kind="ExternalOutput")

    distv = dist.ap().rearrange("(p t) m -> p t m", p=P)      # r = 56p + t
    out_ap = out.ap()

    with tile.TileContext(nc) as tc, ExitStack() as ctx:
        pool_in = ctx.enter_context(tc.tile_pool(name="in", bufs=3))
        pool_neg = ctx.enter_context(tc.tile_pool(name="neg", bufs=3))
        pool_keep = ctx.enter_context(tc.tile_pool(name="keep", bufs=1))
        pool_mm = ctx.enter_context(tc.tile_pool(name="mm", bufs=2))
        pool_ps1 = ctx.enter_context(
            tc.tile_pool(name="ps1", bufs=1, space="PSUM"))
        pool_ps2 = ctx.enter_context(
            tc.tile_pool(name="ps2", bufs=4, space="PSUM"))

        amat_sb = pool_keep.tile([P, 2 * IMG_OUT], bf16)
        nc.scalar.dma_start(amat_sb[:], amat2.ap())

        # preload the sqrt activation table before ScalarE gets busy (the
        # in-loop negates are Copy, which is in every table set, so sqrt
        # stays resident)
        warm = pool_keep.tile([P, 8], f32)
        nc.vector.memset(warm[:], 1.0)
        nc.scalar.activation(warm[:], warm[:], AF.Sqrt)

        top8 = pool_keep.tile([P, 8 * T], bf16)
        top8v = top8[:].rearrange("p (t e) -> p e t", e=8)
        vall = pool_keep.tile([P, 3 * T], f32)   # [v0 | v1 | v2] blocks
        vall3 = vall[:].rearrange("p (e t) -> p e t", e=3)
        dd = pool_keep.tile([P, 2 * T], f32)
        ee = pool_keep.tile([P, 2 * T], f32)
        denom = pool_keep.tile([P, T], f32)
        rec = pool_keep.tile([P, T], f32)
        sval = pool_keep.tile([P, T], bf16)

        def softmin_block(t0, t1):
            # s = v0 / (1 + e^{d1} + e^{d2}),  d_j = v0 - v_j  in [-1, 0].
            # One Sqrt activation (table warm); the exponentials use a
            # cubic Taylor poly on VectorE -- d is the gap between the
            # closest and 2nd/3rd-closest of 4096 distances, |d| <~ 0.05,
            # so the cubic is exact to ~1e-8.
            nc.scalar.activation(vall3[:, :, t0:t1], top8v[:, 0:3, t0:t1],
                                 AF.Sqrt, scale=-1.0)
            v0 = vall[:, t0:t1]
            v1 = vall[:, T + t0:T + t1]
            v2 = vall[:, 2 * T + t0:2 * T + t1]
            d1, d2 = dd[:, t0:t1], dd[:, T + t0:T + t1]
            e1, e2 = ee[:, t0:t1], ee[:, T + t0:T + t1]
            nc.vector.tensor_sub(d1, v0, v1)
            nc.vector.tensor_sub(d2, v0, v2)
            # Horner: e^d - 1 ~ ((d/6 + 1/2)d + 1)d
            for dj, ej in ((d1, e1), (d2, e2)):
                nc.vector.tensor_scalar(ej, dj, 1.0 / 6.0, 0.5,
                                        ALU.mult, ALU.add)
                nc.vector.tensor_mul(ej, ej, dj)
                nc.vector.tensor_scalar_add(ej, ej, 1.0)
                nc.vector.tensor_mul(ej, ej, dj)
            # denom = (e1 + 3) + e2 = 1 + e^{d1} + e^{d2}
            nc.vector.scalar_tensor_tensor(denom[:, t0:t1], e1, 3.0, e2,
                                           ALU.add, ALU.add)
            nc.vector.reciprocal(rec[:, t0:t1], denom[:, t0:t1])
            nc.vector.tensor_mul(sval[:, t0:t1], v0, rec[:, t0:t1])

        def neg_max8(src_ap, dst_ap, cw):
            tneg = pool_neg.tile([P, cw], bf16, tag="neg")
            nc.scalar.mul(tneg[:], src_ap, -1.0)
            nc.vector.max(dst_ap, tneg[:])

        # main stream: 26 pair units (t = 0..51), 32 KiB descriptors
        for u in range(26):
            t0 = 2 * u
            tin = pool_in.tile([P, 2 * M], f32, tag="in")
            dst = tin[:].rearrange("p (k m) -> p k m", k=2)
            src = distv[:, t0:t0 + 2, :]
            if u in SPLIT_UNITS:
                nc.sync.dma_start(dst[0:105], src[0:105])
                nc.sync.dma_start(dst[105:P], src[105:P])
            else:
                nc.sync.dma_start(dst, src)
            for k in range(2):
                t = t0 + k
                neg_max8(tin[:, k * M:(k + 1) * M], top8[:, 8 * t:8 * t + 8], M)
            if u in BLOCKS:
                softmin_block(*BLOCKS[u])

        # stream end on dedicated tiles: DMA never waits on buffer reuse.
        # pair (52, 53) full-width, then t=54 in 2 chunks, t=55 tapering
        # down to 256 so the post-stream drain is a tiny negate+max8.
        tail_pair = pool_keep.tile([P, 2 * M], f32)
        nc.sync.dma_start(tail_pair[:].rearrange("p (k m) -> p k m", k=2),
                          distv[:, 52:54, :])
        for k in range(2):
            t = 52 + k
            neg_max8(tail_pair[:, k * M:(k + 1) * M],
                     top8[:, 8 * t:8 * t + 8], M)

        for t, chunks in ((54, (2048, 2048)), (55, (1024, 1024, 1024, 512, 256, 256))):
            tcol = pool_keep.tile([P, M], f32, tag=f"tcol{t}")
            parts = pool_keep.tile([P, 8 * len(chunks)], bf16,
                                   tag=f"parts{t}")
            off = 0
            for h, cw in enumerate(chunks):
                nc.sync.dma_start(tcol[:, off:off + cw],
                                  distv[:, t, off:off + cw])
                neg_max8(tcol[:, off:off + cw], parts[:, 8 * h:8 * h + 8], cw)
                off += cw
            nc.vector.max(top8[:, 8 * t:8 * t + 8], parts[:])

        softmin_block(52, 56)

        # post: out_i = A @ S_i @ A^T.  mm1 does both images at once
        # (K=112, block-diagonal amat2); sval IS [S_0; S_1] in SBUF.
        ps1 = pool_ps1.tile([IMG_IN, 2 * IMG_OUT], f32)
        nc.tensor.matmul(ps1[:], sval[:], amat_sb[:], start=True, stop=True)
        u1 = pool_mm.tile([IMG_IN, 2 * IMG_OUT], bf16)
        nc.scalar.copy(u1[:, 0:IMG_OUT], ps1[:, 0:IMG_OUT])
        nc.vector.tensor_copy(u1[:, IMG_OUT:2 * IMG_OUT],
                              ps1[:, IMG_OUT:2 * IMG_OUT])
        amat_mm2 = amat_sb[0:IMG_IN, 0:IMG_OUT]      # [56, 224] = A^T
        for i in range(BPC):
            o_all = pool_mm.tile([P, 2 * IMG_OUT], f32)  # [hp, (c w)], ho=2hp+c
            for c in range(2):
                ps2 = pool_ps2.tile([P, IMG_OUT], f32)
                # lhsT free dim = rows ho = c, c+2, ..., c+222 of image i
                lhs = u1[:, i * IMG_OUT + c:(i + 1) * IMG_OUT:2]
                nc.tensor.matmul(ps2[:], lhs, amat_mm2, start=True, stop=True)
                (nc.vector.tensor_copy if c else nc.scalar.copy)(
                    o_all[:, c * IMG_OUT:(c + 1) * IMG_OUT], ps2[:])
            nc.scalar.dma_start(
                out_ap[i].rearrange("(hp c) w -> hp c w", c=2),
                o_all[:].rearrange("p (c w) -> p c w", c=2))

    nc.compile()
    return nc


def _get_nc():
    if "nc" not in _CACHE:
        _CACHE["nc"] = _build()
    return _CACHE["nc"]


def kernel(**inputs) -> np.ndarray:
    from concourse.bass_utils import run_bass_kernel_spmd

    distance = np.ascontiguousarray(np.asarray(inputs["distance"], dtype=np.float32))
    assert distance.shape == (B, HW, M), distance.shape
    amat2 = _amat2()

    nc = _get_nc()
    in_maps = []
    for c in range(N_CORES):
        shard = distance[c * BPC:(c + 1) * BPC].reshape(ROWS, M)
        in_maps.append({"distance": shard, "amat2": amat2})

    trace = bool(int(os.environ.get("KERNEL_TRACE", "0")))
    reps = int(os.environ.get("KERNEL_REPS", "1")) if trace else 1
    times = []
    res = None
    for _ in range(reps):
        try:
            res = run_bass_kernel_spmd(nc, in_maps,
                                       core_ids=list(range(N_CORES)),
                                       trace=trace)
        except ModuleNotFoundError:
            if not trace:
                raise
            trace = False
            res = run_bass_kernel_spmd(nc, in_maps,
                                       core_ids=list(range(N_CORES)),
                                       trace=False)
        if res.exec_time_ns is not None:
            times.append(res.exec_time_ns)
    if times:
        print(f"HW exec times: {times} -> min {min(times)} ns")
        _CACHE["exec_time_ns"] = min(times)
        _CACHE["results"] = res

    outs = [res.results[c]["out"] for c in range(N_CORES)]
    full = np.concatenate(outs, axis=0).reshape(B, 1, IMG_OUT, IMG_OUT)
    return full.astype(np.float32)


# revision 4
# speedup vs baseline: 1.5862x; 1.5862x over previous
"""AnomalyMapGenerator (retrieval kNN) Trainium2 kernel.

reference:  d = sqrt(distance[B, HW, M]); v = 3 smallest of d per row;
            w = softmax(-v); s = w0*v0 -> [B, 56, 56]
            -> bilinear resize to 224x224 -> gaussian blur (sigma=4, reflect).

Strategy (8 NeuronCores, data-parallel over batch, 2 images per core):
  - per core, rows r = b*3136 + hw (6272 rows of 4096 distances), mapped
    r = 49p + t over 128 partitions x 49 columns.  128 descriptors per
    transfer = 8 per SDMA engine, which exactly matches the SBUF AXI
    port map (port q serves partitions {4q..4q+3, 32+4q..32+4q+3}) --
    112-partition transfers measured 33% slower from port collisions.
  - main loop (22 pair units of [128, 2x4096] f32, 32 KiB descriptors =
    measured per-engine throughput sweet spot):
      DMA load -> ScalarE negate -> VectorE max8 (top-8 of -d = 3
      smallest of d; bf16 max8/negate measured ~2x SLOWER, so f32).
  - softmin runs INCREMENTALLY in column blocks behind the stream; only
    columns 42-48 are processed in the tail.  sval is written bf16.
  - stream end: dedicated tiles for t=46..48 (no buffer-recycle
    semaphore stalls) with a chunk taper down to 256 elements, so the
    last-arriving bytes need only a tiny negate+max8.
  - tail: sval [128,49] bf16 -> DRAM smap (98 B/partition) -> per-image
    [56,56] bf16 re-loads on both HWDGE rings in parallel; then
    out_i = A @ S_i @ A^T as bf16 matmuls (4x fp32 PE rate): mm1 K=56,
    then per (image, even/odd row half) K=56 with lhsT free-stride 2 so
    each output image writes 1792 B-contiguous descriptors (rows
    ho = 2*hp + c).  bf16 end-to-end error ~1.6e-3 << 2e-2 tolerance.
"""
import os
import numpy as np

B, HW, M = 16, 3136, 4096
IMG_IN, IMG_OUT, SIGMA = 56, 224, 4.0
N_CORES = 8
BPC = B // N_CORES            # images per core
ROWS = BPC * HW               # 6272
P = 128
T = ROWS // P                 # 49 columns, row r = 49p + t

# SDMA engine n of a transfer handles the n-th contiguous chunk of
# ceil(D/16) descriptors (measured).  Engine 15 is intermittently slower
# than the rest, so a couple of pair units are issued as a [0:120)
# transfer (engines 0-14) plus a [120:128) transfer (engines 0-7),
# shifting those bytes off engine 15.
SPLIT_UNITS = {5, 11}

# softmin blocks emitted mid-stream after the covering unit's max8s
BLOCKS = {6: (0, 14), 13: (14, 28), 20: (28, 42)}

_CACHE = {}


def _resize_mat(in_size: int, out_size: int) -> np.ndarray:
    # jax.image.resize(method='bilinear') upsampling weight matrix [out, in]
    scale = out_size / in_size
    sample_f = (np.arange(out_size, dtype=np.float64) + 0.5) / scale - 0.5
    x = np.abs(sample_f[None, :] - np.arange(in_size, dtype=np.float64)[:, None])
    w = np.maximum(0.0, 1.0 - x)
    total = w.sum(axis=0, keepdims=True)
    w = np.where(np.abs(total) > 1e-8, w / total, 0.0)
    ob = (sample_f < -0.5) | (sample_f > in_size - 0.5)
    w[:, ob] = 0.0
    return w.T


def _gauss_mat(n: int, sigma: float) -> np.ndarray:
    # 1D gaussian conv with reflect padding as a matrix [n, n]
    ksize = 2 * int(4.0 * sigma + 0.5) + 1
    xs = np.arange(ksize, dtype=np.float64) - ksize // 2
    g = np.exp(-(xs * xs) / (2.0 * sigma * sigma))
    g = g / g.sum()
    pad = ksize // 2
    Gm = np.zeros((n, n), dtype=np.float64)
    for o in range(n):
        for k in range(ksize):
            idx = o - pad + k
            if idx < 0:
                idx = -idx
            elif idx > n - 1:
                idx = 2 * (n - 1) - idx
            Gm[o, idx] += g[k]
    return Gm


def _amat_t() -> np.ndarray:
    import ml_dtypes
    A = _gauss_mat(IMG_OUT, SIGMA) @ _resize_mat(IMG_IN, IMG_OUT)  # [224, 56]
    at = A.T.astype(np.float32)                                    # [56, 224]
    return np.ascontiguousarray(at.astype(ml_dtypes.bfloat16))


def _build():
    from contextlib import ExitStack
    import concourse.bass as bass
    import concourse.tile as tile
    from concourse import bacc, mybir

    f32 = mybir.dt.float32
    bf16 = mybir.dt.bfloat16
    AF = mybir.ActivationFunctionType
    ALU = mybir.AluOpType

    nc = bacc.Bacc("TRN2", target_bir_lowering=False, debug=False,
                   enable_asserts=False)
    dist = nc.dram_tensor("distance", [ROWS, M], f32, kind="ExternalInput")
    amat = nc.dram_tensor("amat_t", [IMG_IN, IMG_OUT], bf16, kind="ExternalInput")
    out = nc.dram_tensor("out", [BPC, IMG_OUT, IMG_OUT], f32, kind="ExternalOutput")
    smap = nc.dram_tensor("smap", [ROWS], bf16)  # internal scratch

    distv = dist.ap().rearrange("(p t) m -> p t m", p=P)      # r = 49p + t
    smap_pt = smap.ap().rearrange("(p t) -> p t", p=P)
    smap_img = smap.ap().rearrange("(i h w) -> i h w", i=BPC, h=IMG_IN)
    out_ap = out.ap()

    with tile.TileContext(nc) as tc, ExitStack() as ctx:
        pool_in = ctx.enter_context(tc.tile_pool(name="in", bufs=3))
        pool_neg = ctx.enter_context(tc.tile_pool(name="neg", bufs=3))
        pool_keep = ctx.enter_context(tc.tile_pool(name="keep", bufs=1))
        pool_mm = ctx.enter_context(tc.tile_pool(name="mm", bufs=2))
        pool_ps1 = ctx.enter_context(
            tc.tile_pool(name="ps1", bufs=2, space="PSUM"))
        pool_ps2 = ctx.enter_context(
            tc.tile_pool(name="ps2", bufs=4, space="PSUM"))

        amat_sb = pool_keep.tile([IMG_IN, IMG_OUT], bf16)
        nc.scalar.dma_start(amat_sb[:], amat.ap())

        # preload the sqrt activation table before ScalarE gets busy (the
        # in-loop negates are Copy, which is in every table set, so sqrt
        # stays resident)
        warm = pool_keep.tile([P, 8], f32)
        nc.vector.memset(warm[:], 1.0)
        nc.scalar.activation(warm[:], warm[:], AF.Sqrt)

        top8 = pool_keep.tile([P, 8 * T], f32)
        top8v = top8[:].rearrange("p (t e) -> p e t", e=8)
        vall = pool_keep.tile([P, 3 * T], f32)   # [v0 | v1 | v2] blocks
        vall3 = vall[:].rearrange("p (e t) -> p e t", e=3)
        dd = pool_keep.tile([P, 2 * T], f32)
        ee = pool_keep.tile([P, 2 * T], f32)
        denom = pool_keep.tile([P, T], f32)
        rec = pool_keep.tile([P, T], f32)
        sval = pool_keep.tile([P, T], bf16)

        def softmin_block(t0, t1):
            # s = v0 / (1 + e^{d1} + e^{d2}),  d_j = v0 - v_j  in [-1, 0].
            # One Sqrt activation (table warm); the exponentials use a
            # cubic Taylor poly on VectorE -- d is the gap between the
            # closest and 2nd/3rd-closest of 4096 distances, |d| <~ 0.05,
            # so the cubic is exact to ~1e-8.
            nc.scalar.activation(vall3[:, :, t0:t1], top8v[:, 0:3, t0:t1],
                                 AF.Sqrt, scale=-1.0)
            v0 = vall[:, t0:t1]
            v1 = vall[:, T + t0:T + t1]
            v2 = vall[:, 2 * T + t0:2 * T + t1]
            d1, d2 = dd[:, t0:t1], dd[:, T + t0:T + t1]
            e1, e2 = ee[:, t0:t1], ee[:, T + t0:T + t1]
            nc.vector.tensor_sub(d1, v0, v1)
            nc.vector.tensor_sub(d2, v0, v2)
            # Horner: e^d - 1 ~ ((d/6 + 1/2)d + 1)d
            for dj, ej in ((d1, e1), (d2, e2)):
                nc.vector.tensor_scalar(ej, dj, 1.0 / 6.0, 0.5,
                                        ALU.mult, ALU.add)
                nc.vector.tensor_mul(ej, ej, dj)
                nc.vector.tensor_scalar_add(ej, ej, 1.0)
                nc.vector.tensor_mul(ej, ej, dj)
            # denom = (e1 + 3) + e2 = 1 + e^{d1} + e^{d2}
            nc.vector.scalar_tensor_tensor(denom[:, t0:t1], e1, 3.0, e2,
                                           ALU.add, ALU.add)
            nc.vector.reciprocal(rec[:, t0:t1], denom[:, t0:t1])
            nc.vector.tensor_mul(sval[:, t0:t1], v0, rec[:, t0:t1])

        def neg_max8(src_ap, dst_ap, cw):
            tneg = pool_neg.tile([P, cw], f32, tag="neg")
            nc.scalar.mul(tneg[:], src_ap, -1.0)
            nc.vector.max(dst_ap, tneg[:])

        # main stream: 22 pair units (t = 0..43) + singles t = 44, 45
        units = [(2 * j, 2) for j in range(22)] + [(44, 1), (45, 1)]
        for u, (t0, w) in enumerate(units):
            tin = pool_in.tile([P, w * M], f32, tag="in")
            dst = tin[:].rearrange("p (k m) -> p k m", k=w)
            src = distv[:, t0:t0 + w, :]
            if u in SPLIT_UNITS:
                nc.sync.dma_start(dst[0:120], src[0:120])
                nc.sync.dma_start(dst[120:P], src[120:P])
            else:
                nc.sync.dma_start(dst, src)
            for k in range(w):
                t = t0 + k
                neg_max8(tin[:, k * M:(k + 1) * M], top8[:, 8 * t:8 * t + 8], M)
            if u in BLOCKS:
                softmin_block(*BLOCKS[u])

        # stream end on dedicated tiles: DMA never waits on buffer reuse.
        # t=46 full-width, t=47 in 2 chunks, t=48 tapering down to 256 so
        # the post-stream drain is a tiny negate+max8.
        t46 = pool_keep.tile([P, M], f32)
        nc.sync.dma_start(t46[:], distv[:, 46, :])
        neg_max8(t46[:], top8[:, 8 * 46:8 * 46 + 8], M)

        for t, chunks in ((47, (2048, 2048)), (48, (1024, 1024, 1024, 512, 256, 256))):
            tcol = pool_keep.tile([P, M], f32, tag=f"tcol{t}")
            parts = pool_keep.tile([P, 8 * len(chunks)], f32,
                                   tag=f"parts{t}")
            off = 0
            for h, cw in enumerate(chunks):
                nc.sync.dma_start(tcol[:, off:off + cw],
                                  distv[:, t, off:off + cw])
                neg_max8(tcol[:, off:off + cw], parts[:, 8 * h:8 * h + 8], cw)
                off += cw
            nc.vector.max(top8[:, 8 * t:8 * t + 8], parts[:])

        softmin_block(42, 49)

        # sval -> DRAM (bf16, 98 B/partition), then per-image [56,56]
        # re-loads split across both HWDGE rings to parallelize dispatch
        nc.sync.dma_start(smap_pt, sval[:])
        s_tiles = []
        for i in range(BPC):
            s_i = pool_mm.tile([IMG_IN, IMG_IN], bf16)
            (nc.scalar if i == 0 else nc.sync).dma_start(
                s_i[:], smap_img[i, :, :])
            s_tiles.append(s_i)

        # post: out_i = A @ S_i @ A^T (bf16 PE, 4x fp32 rate)
        ps1s, u1s = [], []
        for i in range(BPC):
            ps1 = pool_ps1.tile([IMG_IN, IMG_OUT], f32)
            # ps1[w', n] = sum_h S[h, w'] * A[n, h]  ==  (A @ S)^T
            nc.tensor.matmul(ps1[:], s_tiles[i][:], amat_sb[:],
                             start=True, stop=True)
            ps1s.append(ps1)
        for i in range(BPC):
            u1 = pool_mm.tile([IMG_IN, IMG_OUT], bf16)
            (nc.scalar.copy if i == 0 else nc.vector.tensor_copy)(
                u1[:], ps1s[i][:])
            u1s.append(u1)
        HP = IMG_OUT // 2
        for i in range(BPC):
            o_all = pool_mm.tile([HP, 2 * IMG_OUT], f32)  # [hp, (c w)], ho=2hp+c
            for c in range(2):
                ps2 = pool_ps2.tile([HP, IMG_OUT], f32)
                # lhsT free dim = rows ho = c, c+2, ..., c+222 of image i
                nc.tensor.matmul(ps2[:], u1s[i][:, c:IMG_OUT:2], amat_sb[:],
                                 start=True, stop=True)
                (nc.vector.tensor_copy if c else nc.scalar.copy)(
                    o_all[:, c * IMG_OUT:(c + 1) * IMG_OUT], ps2[:])
            nc.scalar.dma_start(
                out_ap[i].rearrange("(hp c) w -> hp c w", c=2),
                o_all[:].rearrange("p (c w) -> p c w", c=2))

    nc.compile()
    return nc


def _get_nc():
    if "nc" not in _CACHE:
        _CACHE["nc"] = _build()
    return _CACHE["nc"]


def kernel(**inputs) -> np.ndarray:
    from concourse.bass_utils import run_bass_kernel_spmd

    distance = np.ascontiguousarray(np.asarray(inputs["distance"], dtype=np.float32))
    assert distance.shape == (B, HW, M), distance.shape
    amat_t = _amat_t()

    nc = _get_nc()
    in_maps = []
    for c in range(N_CORES):
        shard = distance[c * BPC:(c + 1) * BPC].reshape(ROWS, M)
        in_maps.append({"distance": shard, "amat_t": amat_t})

    trace = bool(int(os.environ.get("KERNEL_TRACE", "0")))
    reps = int(os.environ.get("KERNEL_REPS", "1")) if trace else 1
    times = []
    res = None
    for _ in range(reps):
        try:
            res = run_bass_kernel_spmd(nc, in_maps,
                                       core_ids=list(range(N_CORES)),
                                       trace=trace)
        except ModuleNotFoundError:
            if not trace:
                raise
            trace = False
            res = run_bass_kernel_spmd(nc, in_maps,
                                       core_ids=list(range(N_CORES)),
                                       trace=False)
        if res.exec_time_ns is not None:
            times.append(res.exec_time_ns)
    if times:
        print(f"HW exec times: {times} -> min {min(times)} ns")
        _CACHE["exec_time_ns"] = min(times)
        _CACHE["results"] = res

    outs = [res.results[c]["out"] for c in range(N_CORES)]
    full = np.concatenate(outs, axis=0).reshape(B, 1, IMG_OUT, IMG_OUT)
    return full.astype(np.float32)


# revision 6
# speedup vs baseline: 1.5944x; 1.0051x over previous
"""AnomalyMapGenerator (retrieval kNN) Trainium2 kernel.

reference:  d = sqrt(distance[B, HW, M]); v = 3 smallest of d per row;
            w = softmax(-v); s = w0*v0 -> [B, 56, 56]
            -> bilinear resize to 224x224 -> gaussian blur (sigma=4, reflect).

Strategy (8 NeuronCores, data-parallel over batch, 2 images per core):
  - per core, rows r = b*3136 + hw (6272 rows of 4096 distances), mapped
    r = 49p + t over 128 partitions x 49 columns.  128 descriptors per
    transfer = 8 per SDMA engine, which exactly matches the SBUF AXI
    port map (port q serves partitions {4q..4q+3, 32+4q..32+4q+3}) --
    112-partition transfers measured 33% slower from port collisions.
  - main loop (22 pair units of [128, 2x4096] f32, 32 KiB descriptors =
    measured per-engine throughput sweet spot):
      DMA load -> ScalarE negate -> VectorE max8 (top-8 of -d = 3
      smallest of d; bf16 max8/negate measured ~2x SLOWER, so f32).
  - softmin runs INCREMENTALLY in column blocks behind the stream; only
    columns 42-48 are processed in the tail.  sval is written bf16.
  - stream end: dedicated tiles for t=46..48 (no buffer-recycle
    semaphore stalls) with a chunk taper down to 256 elements, so the
    last-arriving bytes need only a tiny negate+max8.
  - tail: sval [128,49] bf16 -> DRAM smap (98 B/partition) -> per-image
    [56,56] bf16 re-loads on both HWDGE rings in parallel; then
    out_i = A @ S_i @ A^T as bf16 matmuls (4x fp32 PE rate): mm1 K=56,
    then per (image, even/odd row half) K=56 with lhsT free-stride 2 so
    each output image writes 1792 B-contiguous descriptors (rows
    ho = 2*hp + c).  bf16 end-to-end error ~1.6e-3 << 2e-2 tolerance.
"""
import os
import numpy as np

B, HW, M = 16, 3136, 4096
IMG_IN, IMG_OUT, SIGMA = 56, 224, 4.0
N_CORES = 8
BPC = B // N_CORES            # images per core
ROWS = BPC * HW               # 6272
P = 128
T = ROWS // P                 # 49 columns, row r = 49p + t

# SDMA engine n of a transfer handles the n-th contiguous chunk of
# ceil(D/16) descriptors (measured).  Engine 15 is intermittently slower
# than the rest, so a couple of pair units are issued as a [0:120)
# transfer (engines 0-14) plus a [120:128) transfer (engines 0-7),
# shifting those bytes off engine 15.
SPLIT_UNITS = {5, 11}

# softmin blocks emitted mid-stream after the covering unit's max8s;
# each block's sval columns are also written to DRAM smap mid-stream
# (per-partition contiguous: smap is r-flat, r = 49p + t), so the tail
# only writes columns [40:49).
BLOCKS = {6: (0, 14), 13: (14, 28), 19: (28, 40)}

_CACHE = {}


def _resize_mat(in_size: int, out_size: int) -> np.ndarray:
    # jax.image.resize(method='bilinear') upsampling weight matrix [out, in]
    scale = out_size / in_size
    sample_f = (np.arange(out_size, dtype=np.float64) + 0.5) / scale - 0.5
    x = np.abs(sample_f[None, :] - np.arange(in_size, dtype=np.float64)[:, None])
    w = np.maximum(0.0, 1.0 - x)
    total = w.sum(axis=0, keepdims=True)
    w = np.where(np.abs(total) > 1e-8, w / total, 0.0)
    ob = (sample_f < -0.5) | (sample_f > in_size - 0.5)
    w[:, ob] = 0.0
    return w.T


def _gauss_mat(n: int, sigma: float) -> np.ndarray:
    # 1D gaussian conv with reflect padding as a matrix [n, n]
    ksize = 2 * int(4.0 * sigma + 0.5) + 1
    xs = np.arange(ksize, dtype=np.float64) - ksize // 2
    g = np.exp(-(xs * xs) / (2.0 * sigma * sigma))
    g = g / g.sum()
    pad = ksize // 2
    Gm = np.zeros((n, n), dtype=np.float64)
    for o in range(n):
        for k in range(ksize):
            idx = o - pad + k
            if idx < 0:
                idx = -idx
            elif idx > n - 1:
                idx = 2 * (n - 1) - idx
            Gm[o, idx] += g[k]
    return Gm


def _amat_t() -> np.ndarray:
    import ml_dtypes
    A = _gauss_mat(IMG_OUT, SIGMA) @ _resize_mat(IMG_IN, IMG_OUT)  # [224, 56]
    at = A.T.astype(np.float32)                                    # [56, 224]
    return np.ascontiguousarray(at.astype(ml_dtypes.bfloat16))


def _build():
    from contextlib import ExitStack
    import concourse.bass as bass
    import concourse.tile as tile
    from concourse import bacc, mybir

    f32 = mybir.dt.float32
    bf16 = mybir.dt.bfloat16
    AF = mybir.ActivationFunctionType
    ALU = mybir.AluOpType

    nc = bacc.Bacc("TRN2", target_bir_lowering=False, debug=False,
                   enable_asserts=False)
    dist = nc.dram_tensor("distance", [ROWS, M], f32, kind="ExternalInput")
    amat = nc.dram_tensor("amat_t", [IMG_IN, IMG_OUT], bf16, kind="ExternalInput")
    out = nc.dram_tensor("out", [BPC, IMG_OUT, IMG_OUT], f32, kind="ExternalOutput")
    smap = nc.dram_tensor("smap", [ROWS], bf16)  # internal scratch

    distv = dist.ap().rearrange("(p t) m -> p t m", p=P)      # r = 49p + t
    smap_pt = smap.ap().rearrange("(p t) -> p t", p=P)
    smap_img = smap.ap().rearrange("(i h w) -> i h w", i=BPC, h=IMG_IN)
    out_ap = out.ap()

    with tile.TileContext(nc) as tc, ExitStack() as ctx:
        pool_in = ctx.enter_context(tc.tile_pool(name="in", bufs=3))
        pool_neg = ctx.enter_context(tc.tile_pool(name="neg", bufs=3))
        pool_keep = ctx.enter_context(tc.tile_pool(name="keep", bufs=1))
        pool_mm = ctx.enter_context(tc.tile_pool(name="mm", bufs=2))
        pool_ps1 = ctx.enter_context(
            tc.tile_pool(name="ps1", bufs=2, space="PSUM"))
        pool_ps2 = ctx.enter_context(
            tc.tile_pool(name="ps2", bufs=4, space="PSUM"))

        amat_sb = pool_keep.tile([IMG_IN, IMG_OUT], bf16)
        nc.scalar.dma_start(amat_sb[:], amat.ap())

        # preload the sqrt activation table before ScalarE gets busy (the
        # in-loop negates are Copy, which is in every table set, so sqrt
        # stays resident)
        warm = pool_keep.tile([P, 8], f32)
        nc.vector.memset(warm[:], 1.0)
        nc.scalar.activation(warm[:], warm[:], AF.Sqrt)

        top8 = pool_keep.tile([P, 8 * T], f32)
        top8v = top8[:].rearrange("p (t e) -> p e t", e=8)
        vall = pool_keep.tile([P, 3 * T], f32)   # [v0 | v1 | v2] blocks
        vall3 = vall[:].rearrange("p (e t) -> p e t", e=3)
        dd = pool_keep.tile([P, 2 * T], f32)
        ee = pool_keep.tile([P, 2 * T], f32)
        denom = pool_keep.tile([P, T], f32)
        rec = pool_keep.tile([P, T], f32)
        sval = pool_keep.tile([P, T], bf16)

        def softmin_block(t0, t1):
            # s = v0 / (1 + e^{d1} + e^{d2}),  d_j = v0 - v_j  in [-1, 0].
            # One Sqrt activation (table warm); the exponentials use a
            # cubic Taylor poly on VectorE -- d is the gap between the
            # closest and 2nd/3rd-closest of 4096 distances, |d| <~ 0.05,
            # so the cubic is exact to ~1e-8.
            nc.scalar.activation(vall3[:, :, t0:t1], top8v[:, 0:3, t0:t1],
                                 AF.Sqrt, scale=-1.0)
            v0 = vall[:, t0:t1]
            v1 = vall[:, T + t0:T + t1]
            v2 = vall[:, 2 * T + t0:2 * T + t1]
            d1, d2 = dd[:, t0:t1], dd[:, T + t0:T + t1]
            e1, e2 = ee[:, t0:t1], ee[:, T + t0:T + t1]
            nc.vector.tensor_sub(d1, v0, v1)
            nc.vector.tensor_sub(d2, v0, v2)
            # Horner: e^d - 1 ~ ((d/6 + 1/2)d + 1)d
            for dj, ej in ((d1, e1), (d2, e2)):
                nc.vector.tensor_scalar(ej, dj, 1.0 / 6.0, 0.5,
                                        ALU.mult, ALU.add)
                nc.vector.tensor_mul(ej, ej, dj)
                nc.vector.tensor_scalar_add(ej, ej, 1.0)
                nc.vector.tensor_mul(ej, ej, dj)
            # denom = (e1 + 3) + e2 = 1 + e^{d1} + e^{d2}
            nc.vector.scalar_tensor_tensor(denom[:, t0:t1], e1, 3.0, e2,
                                           ALU.add, ALU.add)
            nc.vector.reciprocal(rec[:, t0:t1], denom[:, t0:t1])
            nc.vector.tensor_mul(sval[:, t0:t1], v0, rec[:, t0:t1])

        def neg_max8(src_ap, dst_ap, cw):
            tneg = pool_neg.tile([P, cw], f32, tag="neg")
            nc.scalar.mul(tneg[:], src_ap, -1.0)
            nc.vector.max(dst_ap, tneg[:])

        def chunked_col(t, tcol, chunks):
            # per-chunk negate+max8 pipelines with the chunk arrivals, so
            # full-width columns never serialize ~6 us of drain at the end
            parts = pool_keep.tile([P, 8 * len(chunks)], f32, tag=f"parts{t}")
            off = 0
            for h, cw in enumerate(chunks):
                nc.sync.dma_start(tcol[:, off:off + cw],
                                  distv[:, t, off:off + cw])
                neg_max8(tcol[:, off:off + cw], parts[:, 8 * h:8 * h + 8], cw)
                off += cw
            nc.vector.max(top8[:, 8 * t:8 * t + 8], parts[:])

        # main stream: 22 pair units (t = 0..43)
        for u in range(22):
            t0 = 2 * u
            tin = pool_in.tile([P, 2 * M], f32, tag="in")
            dst = tin[:].rearrange("p (k m) -> p k m", k=2)
            src = distv[:, t0:t0 + 2, :]
            if u in SPLIT_UNITS:
                nc.sync.dma_start(dst[0:120], src[0:120])
                nc.sync.dma_start(dst[120:P], src[120:P])
            else:
                nc.sync.dma_start(dst, src)
            for k in range(2):
                t = t0 + k
                neg_max8(tin[:, k * M:(k + 1) * M], top8[:, 8 * t:8 * t + 8], M)
            if u in BLOCKS:
                softmin_block(*BLOCKS[u])
                t0b, t1b = BLOCKS[u]
                nc.sync.dma_start(smap_pt[:, t0b:t1b], sval[:, t0b:t1b])

        # stream end: singles t=44,45 in pool tiles, t=46..48 on dedicated
        # tiles (DMA never waits on buffer reuse), all chunked
        for t in (44, 45):
            tcol = pool_in.tile([P, M], f32, tag="in")
            chunked_col(t, tcol, (2048, 2048))
        for t, chunks in ((46, (2048, 2048)), (47, (2048, 2048)),
                          (48, (1024, 1024, 1024, 512, 256, 256))):
            tcol = pool_keep.tile([P, M], f32, tag=f"tcol{t}")
            chunked_col(t, tcol, chunks)

        softmin_block(40, 49)

        # tail sval -> DRAM (bf16, 18 B/partition), then per-image [56,56]
        # re-loads split across both HWDGE rings to parallelize dispatch
        nc.sync.dma_start(smap_pt[:, 40:49], sval[:, 40:49])
        s_tiles = []
        for i in range(BPC):
            s_i = pool_mm.tile([IMG_IN, IMG_IN], bf16)
            (nc.scalar if i == 0 else nc.sync).dma_start(
                s_i[:], smap_img[i, :, :])
            s_tiles.append(s_i)

        # post: out_i = A @ S_i @ A^T (bf16 PE, 4x fp32 rate)
        ps1s, u1s = [], []
        for i in range(BPC):
            ps1 = pool_ps1.tile([IMG_IN, IMG_OUT], f32)
            # ps1[w', n] = sum_h S[h, w'] * A[n, h]  ==  (A @ S)^T
            nc.tensor.matmul(ps1[:], s_tiles[i][:], amat_sb[:],
                             start=True, stop=True)
            ps1s.append(ps1)
        for i in range(BPC):
            u1 = pool_mm.tile([IMG_IN, IMG_OUT], bf16)
            (nc.scalar.copy if i == 0 else nc.vector.tensor_copy)(
                u1[:], ps1s[i][:])
            u1s.append(u1)
        HP = IMG_OUT // 2
        for i in range(BPC):
            o_all = pool_mm.tile([HP, 2 * IMG_OUT], f32)  # [hp, (c w)], ho=2hp+c
            for c in range(2):
                ps2 = pool_ps2.tile([HP, IMG_OUT], f32)
                # lhsT free dim = rows ho = c, c+2, ..., c+222 of image i
                nc.tensor.matmul(ps2[:], u1s[i][:, c:IMG_OUT:2], amat_sb[:],
                                 start=True, stop=True)
                (nc.vector.tensor_copy if c else nc.scalar.copy)(
                    o_all[:, c * IMG_OUT:(c + 1) * IMG_OUT], ps2[:])
            nc.scalar.dma_start(
                out_ap[i].rearrange("(hp c) w -> hp c w", c=2),
                o_all[:].rearrange("p (c w) -> p c w", c=2))

    nc.compile()
    return nc


def _get_nc():
    if "nc" not in _CACHE:
        _CACHE["nc"] = _build()
    return _CACHE["nc"]


def kernel(**inputs) -> np.ndarray:
    from concourse.bass_utils import run_bass_kernel_spmd

    distance = np.ascontiguousarray(np.asarray(inputs["distance"], dtype=np.float32))
    assert distance.shape == (B, HW, M), distance.shape
    amat_t = _amat_t()

    nc = _get_nc()
    in_maps = []
    for c in range(N_CORES):
        shard = distance[c * BPC:(c + 1) * BPC].reshape(ROWS, M)
        in_maps.append({"distance": shard, "amat_t": amat_t})

    trace = bool(int(os.environ.get("KERNEL_TRACE", "0")))
    reps = int(os.environ.get("KERNEL_REPS", "1")) if trace else 1
    times = []
    res = None
    for _ in range(reps):
        try:
            res = run_bass_kernel_spmd(nc, in_maps,
                                       core_ids=list(range(N_CORES)),
                                       trace=trace)
        except ModuleNotFoundError:
            if not trace:
                raise
            trace = False
            res = run_bass_kernel_spmd(nc, in_maps,
                                       core_ids=list(range(N_CORES)),
                                       trace=False)
        if res.exec_time_ns is not None:
            times.append(res.exec_time_ns)
    if times:
        print(f"HW exec times: {times} -> min {min(times)} ns")
        _CACHE["exec_time_ns"] = min(times)
        _CACHE["results"] = res

    outs = [res.results[c]["out"] for c in range(N_CORES)]
    full = np.concatenate(outs, axis=0).reshape(B, 1, IMG_OUT, IMG_OUT)
    return full.astype(np.float32)
